# revision 72
# baseline (speedup 1.0000x reference)
"""DialogueGCN forward as a Bass/Tile kernel on 8 TRN2 NeuronCores (v2).

Sharding: data-parallel over dialogues (batch). Each core owns 32 contiguous
dialogues; edges never cross dialogues so all graph aggregation is local.

The edge window |u-t| <= 10 makes every 300x300 attention / aggregation
matrix banded; all banded matmuls are tiled u-major (3 seq tiles of
128/128/44 rows) with a t-window of uk+20 columns per tile, so the PE only
touches the band (+pad) instead of the dense 300 columns. All matmul
operands are bf16 (rel tolerance 2e-2; bf16 keeps DVE 2x/4x modes live).

Per-dialogue math (u = source utterance, t = target, band |u-t|<=10):
  P_k[u,t]    = exp((W_att^T x^T)[u, t-window_k])          (banded)
  sums[u]     = sum_t P*win  (stt accum);  rc = 1/sums
  S_{s,dd,k}  = P * (msk_s*rc)[u] * dir_dd                 (banded)
  G_{s,dd}    = sum_u x[u,:]^T S_{s,dd}[u,t]               (banded matmul;
                psum column overlap handled by segment splits)
  ph1_d       = sum_{s,dd} (w8[s,0,dd]-w8[s,1,dd])^T G_{s,dd}
  ph1_b       = root^T x^T + sum_{s,dd} w8[s,1,dd]^T G_{s,dd}
  h1f         = ph1_b + bias_r + tmb * ph1_d   (tmb = [spk_tgt==0] mask)
  qt[u,h]     = (h1^T W2);  ph2 = W1^T h1 + banded-win-agg(qt) + b_gc
  hid         = relu(Wlind^T x^T + Wlinh^T h2 + b_lin)
  plgT[u,c]   = (hid^T W_fc)[u,c] + b_fc (rank-1 matmul);
  out         = log_softmax rows (batched per 8-dialogue chunk)
"""

import os

import numpy as np
import ml_dtypes

import concourse.bass as bass
import concourse.mybir as mybir
import concourse.tile as tile
from concourse import bass_utils

SEQ, BATCH, D, H, NCLS = 300, 256, 200, 128, 6
WP = WF = 10
NCORES = 8
BPC = BATCH // NCORES  # dialogues per core
F32 = mybir.dt.float32
BF = mybir.dt.bfloat16
F8 = mybir.dt.float8e4
BF_NP = ml_dtypes.bfloat16
F8_NP = ml_dtypes.float8_e4m3
H1S = 64.0  # h1-path scale so w8 lands in fp8e4 normal range

# u-major banded tiles: (u0, uk, c0, wk) with t-window [c0, c0+wk)
K = []
for _u0 in (0, 128, 256):
    _uk = min(128, SEQ - _u0)
    _c0 = max(0, _u0 - WP)
    _c1 = min(SEQ, _u0 + _uk + WF)
    K.append((_u0, _uk, _c0, _c1 - _c0))
WPAD = 148  # >= max wk
# psum column offset of each tile's window inside the shared pscale bank
POFF = [0]
for _k in range(1, 3):
    POFF.append(POFF[-1] + K[_k - 1][3])

# dialogues per log-softmax / output-DMA chunk; smaller final chunks keep
# the end-of-program tail short
CHUNKS = [8, 8, 8, 5, 3]
CHOFF = [0]
for _c in CHUNKS:
    CHOFF.append(CHOFF[-1] + _c)
NCH = len(CHUNKS)


def _chunk_of(b):
    for ci in range(NCH):
        if b < CHOFF[ci + 1]:
            return ci, b - CHOFF[ci]
    raise ValueError(b)

_CACHE = {}


def _split_multiwaits(nc, max_waits=1):
    """walrus in this container rejects >1 sem wait on an instruction
    ("Too many sync wait commands"); hoist extras onto preceding NOPs."""
    n = 0
    for f in nc.m.functions:
        for b in f.blocks:
            newlist = []
            changed = False
            for ins in b.instructions:
                si = ins.sync_info
                if si is not None and si.on_wait is not None and len(si.on_wait) > max_waits:
                    waits = list(si.on_wait)
                    for w in waits[max_waits:]:
                        n += 1
                        nop = mybir.InstNoOp(name=f"waitsplit-{n}", ins=[], outs=[])
                        nop.engine = ins.engine
                        nop.sync_info = mybir.SyncInfo(on_wait=[w], on_update=[])
                        newlist.append(nop)
                        nc.inst_map[nop.name] = nop
                    ins.sync_info = mybir.SyncInfo(
                        on_wait=waits[:max_waits],
                        on_update=list(si.on_update) if si.on_update else [],
                    )
                    changed = True
                newlist.append(ins)
            if changed:
                b.instructions = newlist
    return n


def _g_segments(k):
    """Column segments (j0, j1, start) in window-local coords for banded psum
    accumulation: columns overlapping the previous tile's window accumulate,
    fresh columns start a new psum region."""
    u0, uk, c0, wk = K[k]
    if k == 0:
        return [(0, wk, True)]
    pc1 = K[k - 1][2] + K[k - 1][3]
    ov = pc1 - c0
    return [(0, ov, False), (ov, wk, True)]


def _build_program():
    nc = bass.Bass("TRN2", num_devices=NCORES)

    ap = {}

    def din(name, shape, dt=BF):
        ap[name] = nc.dram_tensor(name, shape, dt, kind="ExternalInput").ap()

    din("xt2", (2, 100, BPC * SEQ))          # x^T, d-major chunks
    din("xn0", (128, BPC * D))               # x, seq tile 0
    din("xn1", (128, BPC * D))
    din("xn2", (44, BPC * D))
    din("mskp", (128, BPC * 6), F32)         # speaker one-hot / (b,k,s)
    din("tmb", (BPC, SEQ))                   # [spk==0] per (b, t)
    din("wattc", (2, 100, SEQ))
    din("w8d", (2, 100, 4 * H), F8)          # (w8[s,0,dd]-w8[s,1,dd]) * H1S
    din("w8b", (2, 100, 4 * H), F8)          # w8[s,1,dd] * H1S
    din("rootc", (100, 2 * H))
    din("wlindc", (100, 2 * H))
    din("w1c", (H, H))
    din("w2c", (H, H))
    din("wlinhc", (H, H))
    din("wfcc", (H, NCLS))
    din("onesc", (1, H))
    din("bfcrow", (1, 3 * NCLS))
    din("d0b", (3, 128, WPAD))
    din("d1b", (3, 128, WPAD))
    din("wnb", (3, 128, WPAD))
    din("brc", (H, 1), F32)
    din("bgc", (H, 1), F32)
    din("blc", (H, 1), F32)
    out = nc.dram_tensor("out", (BPC * SEQ, NCLS), F32, kind="ExternalOutput").ap()
    if os.environ.get("BASS_DEBUG_TAPS"):
        ap["dbg_h1f"] = nc.dram_tensor("dbg_h1f", (BPC, H, SEQ), BF,
                                       kind="ExternalOutput").ap()
        ap["dbg_g0"] = nc.dram_tensor("dbg_g0", (BPC, 100, 600), F8,
                                      kind="ExternalOutput").ap()
        ap["dbg_s00"] = nc.dram_tensor("dbg_s00", (BPC, 128, WPAD), BF,
                                       kind="ExternalOutput").ap()
        ap["dbg_rc"] = nc.dram_tensor("dbg_rc", (BPC, 128, 3), F32,
                                      kind="ExternalOutput").ap()
        ap["dbg_xs0"] = nc.dram_tensor("dbg_xs0", (BPC, 128, 2 * D), BF,
                                       kind="ExternalOutput").ap()
        ap["dbg_h2"] = nc.dram_tensor("dbg_h2", (BPC, H, SEQ), BF,
                                      kind="ExternalOutput").ap()
        ap["dbg_qts"] = nc.dram_tensor("dbg_qts", (BPC, 128, 3 * H), BF,
                                       kind="ExternalOutput").ap()
        ap["dbg_hid"] = nc.dram_tensor("dbg_hid", (BPC, H, SEQ), BF,
                                       kind="ExternalOutput").ap()

    repeat = int(os.environ.get("BASS_REPEAT", "1"))
    from contextlib import ExitStack
    with tile.TileContext(nc) as tc:
        with ExitStack() as ctx:
            pools = _mk_pools(tc, ctx)
            if repeat > 1:
                with tc.For_i(0, repeat, 1):
                    _body(nc, tc, ap, out, pools)
            else:
                _body(nc, tc, ap, out, pools)

    _split_multiwaits(nc)
    return nc


def _mk_pools(tc, ctx):
    return dict(
        cpool=ctx.enter_context(tc.tile_pool(name="const", bufs=1)),
        pp=ctx.enter_context(tc.tile_pool(name="pp", bufs=3)),
        ss=ctx.enter_context(tc.tile_pool(name="ss", bufs=3)),
        wk=ctx.enter_context(tc.tile_pool(name="wk", bufs=2)),
        gsb=ctx.enter_context(tc.tile_pool(name="gsb", bufs=3)),
        ps_p=ctx.enter_context(tc.tile_pool(name="ps_p", bufs=1, space="PSUM")),
        ps_g=ctx.enter_context(tc.tile_pool(name="ps_g", bufs=3, space="PSUM")),
        ps_b=ctx.enter_context(tc.tile_pool(name="ps_b", bufs=4, space="PSUM")),
    )


PAIRS = [(0, 0), (0, 1), (1, 0), (1, 1)]  # (s, dd)


def _body(nc, tc, ap, out, pools):
    cpool = pools["cpool"]
    pp = pools["pp"]
    ss = pools["ss"]
    wk = pools["wk"]
    gsb = pools["gsb"]
    ps_p = pools["ps_p"]
    ps_g = pools["ps_g"]
    ps_b = pools["ps_b"]

    AF = mybir.ActivationFunctionType
    OP = mybir.AluOpType
    AX = mybir.AxisListType

    # ---- resident inputs / constants ----
    # All input loads ordered by first use; the serial DMA stream is the
    # startup critical path, so the big x loads stream in quarters and the
    # per-stage weights slot in between.
    c_watt = cpool.tile([100, 2 * SEQ], BF, name="c_watt")
    c_xt = cpool.tile([100, 2 * BPC * SEQ], BF, name="c_xt")
    c_mskp = cpool.tile([128, BPC * 6], F32, name="c_mskp")
    c_xn = [cpool.tile([128, BPC * D], BF, name=f"c_xn{k}") for k in range(3)]
    c_dir = {nm: cpool.tile([128, 3 * WPAD], BF, name=f"c_{nm}")
             for nm in ("wnb", "d0b", "d1b")}
    c_w8d = cpool.tile([100, 2 * 4 * H], F8, name="c_w8d")
    c_w8b = cpool.tile([100, 2 * 4 * H], F8, name="c_w8b")
    c_root = cpool.tile([100, 2 * H], BF, name="c_root")
    c_wlind = cpool.tile([100, 2 * H], BF, name="c_wlind")
    c_w1 = cpool.tile([H, H], BF, name="c_w1")
    c_w2 = cpool.tile([H, H], BF, name="c_w2")
    c_wlinh = cpool.tile([H, H], BF, name="c_wlinh")
    c_wfc = cpool.tile([H, NCLS], BF, name="c_wfc")
    c_ones = cpool.tile([1, H], BF, name="c_ones")
    c_bfc = cpool.tile([1, 3 * NCLS], BF, name="c_bfc")
    c_brc = cpool.tile([H, 1], F32, name="c_brc")
    c_bgc = cpool.tile([H, 1], F32, name="c_bgc")
    c_blc = cpool.tile([H, 1], F32, name="c_blc")
    c_tmb = cpool.tile([128, BPC * SEQ], BF, name="c_tmb")

    def xtc(ch):
        return c_xt[:, ch * BPC * SEQ:(ch + 1) * BPC * SEQ]

    def _ld_xt(q, nq=4):
        w = BPC * SEQ // nq
        for ch in range(2):
            nc.sync.dma_start(
                c_xt[:, ch * BPC * SEQ + q * w: ch * BPC * SEQ + (q + 1) * w],
                ap["xt2"][ch, :, q * w:(q + 1) * w])

    def _ld_xn(q, nq=4):
        w = BPC * D // nq
        for k in range(3):
            nc.sync.dma_start(c_xn[k][:K[k][1], q * w:(q + 1) * w],
                              ap[f"xn{k}"][:K[k][1], q * w:(q + 1) * w])

    nc.sync.dma_start(c_watt.rearrange("p (c u) -> p c u", c=2),
                      ap["wattc"].transpose([1, 0, 2]))
    _ld_xt(0)
    nc.sync.dma_start(c_mskp[:], ap["mskp"][:])
    nc.sync.dma_start(c_dir["wnb"].rearrange("p (k x) -> p k x", k=3),
                      ap["wnb"].transpose([1, 0, 2]))
    _ld_xn(0)
    for nm in ("d0b", "d1b"):
        nc.sync.dma_start(c_dir[nm].rearrange("p (k x) -> p k x", k=3),
                          ap[nm].transpose([1, 0, 2]))
    nc.sync.dma_start(c_w8d.rearrange("p (c x) -> p c x", c=2),
                      ap["w8d"].transpose([1, 0, 2]))
    nc.sync.dma_start(c_w8b.rearrange("p (c x) -> p c x", c=2),
                      ap["w8b"].transpose([1, 0, 2]))
    nc.sync.dma_start(c_root[:], ap["rootc"][:])
    nc.sync.dma_start(c_brc[:], ap["brc"][:])
    _ld_xt(1)
    _ld_xn(1)

    def _ld_tmb(q, nq=4):
        w = BPC // nq
        nc.sync.dma_start(
            c_tmb.rearrange("p (b t) -> p b t", b=BPC)[:, q * w:(q + 1) * w, :],
            ap["tmb"][q * w:(q + 1) * w].unsqueeze(0).partition_broadcast(128))

    _ld_tmb(0)
    nc.sync.dma_start(c_w1[:], ap["w1c"][:])
    nc.sync.dma_start(c_w2[:], ap["w2c"][:])
    nc.sync.dma_start(c_bgc[:], ap["bgc"][:])
    nc.sync.dma_start(c_wlind[:], ap["wlindc"][:])
    nc.sync.dma_start(c_wlinh[:], ap["wlinhc"][:])
    nc.sync.dma_start(c_blc[:], ap["blc"][:])
    nc.sync.dma_start(c_wfc[:], ap["wfcc"][:])
    nc.sync.dma_start(c_ones[:], ap["onesc"][:])
    nc.sync.dma_start(c_bfc[:], ap["bfcrow"][:])
    _ld_xt(2)
    _ld_xn(2)
    _ld_tmb(1)
    _ld_xt(3)
    _ld_xn(3)
    _ld_tmb(2)
    _ld_tmb(3)

    losb = [cpool.tile([128, CHUNKS[c] * 18], BF, name=f"losb{c}")
            for c in range(NCH)]
    osb = [cpool.tile([128, CHUNKS[c] * 18], F32, name=f"osb{c}")
           for c in range(NCH)]

    ov = out.rearrange("(b t) c -> b t c", b=BPC)

    # ---------------- per-dialogue stage emitters ----------------

    def em_front(b):
        """pscale matmuls (all 6 first), one merged exp, sums chain, and all
        12 scaled score tiles for dialogue b."""
        ps = ps_p.tile([128, 512], F32, name="psc", tag="psc")
        for k, (u0, uk, c0, wkk) in enumerate(K):
            for ch in range(2):
                nc.tensor.matmul(
                    ps[:uk, POFF[k]:POFF[k] + wkk],
                    c_watt[:, ch * SEQ + u0: ch * SEQ + u0 + uk],
                    xtc(ch)[:, b * SEQ + c0: b * SEQ + c0 + wkk],
                    start=(k == 0 and ch == 0), stop=(k == 2 and ch == 1))
        # one exp over all three windows (rows past uk are garbage, unused)
        pt = pp.tile([128, 344], BF, name="pt", tag="pt")
        TW = POFF[2] + K[2][3]
        nc.scalar.activation(pt[:, :TW], ps[:, :TW], AF.Exp)

        acc = wk.tile([128, 3], F32, name="acc", tag="acc")
        nc.gpsimd.memset(acc[:], 1.0)
        for k, (u0, uk, c0, wkk) in enumerate(K):
            sw = wk.tile([128, WPAD], BF, name="swin", tag="swin")
            nc.vector.scalar_tensor_tensor(
                sw[:uk, :wkk], pt[:uk, POFF[k]:POFF[k] + wkk], 1.0,
                c_dir["wnb"][:uk, k * WPAD:k * WPAD + wkk],
                op0=OP.mult, op1=OP.mult,
                accum_out=acc[:uk, k:k + 1])
        rc = wk.tile([128, 3], F32, name="rc", tag="rc")
        nc.vector.reciprocal(rc[:], acc[:])
        # dir-masked scores: dd0 on DVE, dd1 on Pool; the 1/sums renorm folds
        # into xs below, so these depend only on exp
        stiles = []
        for dd in range(2):
            eng = nc.vector if dd == 0 else nc.gpsimd
            row = []
            for k, (u0, uk, c0, wkk) in enumerate(K):
                st = ss.tile([128, WPAD], BF, name=f"s{dd}{k}", tag=f"s{dd}{k}")
                eng.tensor_tensor(
                    st[:uk, :wkk], pt[:uk, POFF[k]:POFF[k] + wkk],
                    c_dir[f"d{dd}b"][:uk, k * WPAD:k * WPAD + wkk],
                    op=OP.mult)
                row.append(st)
            stiles.append(row)
        if "dbg_rc" in ap:
            nc.sync.dma_start(ap["dbg_rc"][b], rc[:])
            nc.sync.dma_start(ap["dbg_s00"][b], stiles[0][0][:])
        # x * (speaker mask * 1/sums) per contraction row: carries both the
        # speaker selection and the renormalization of the scores
        rm = wk.tile([128, 6], F32, name="rm", tag="rm")
        for k in range(3):
            nc.gpsimd.tensor_scalar_mul(
                rm[:, 2 * k:2 * k + 2],
                c_mskp[:, b * 6 + 2 * k: b * 6 + 2 * k + 2],
                rc[:, k:k + 1])
        xs = []
        for k, (u0, uk, c0, wkk) in enumerate(K):
            xk = wk.tile([128, 2 * D], BF, name=f"xs{k}", tag=f"xs{k}")
            for s in range(2):
                eng = nc.vector if s == 0 else nc.gpsimd
                eng.tensor_scalar_mul(
                    xk[:uk, s * D:(s + 1) * D],
                    c_xn[k][:uk, b * D:(b + 1) * D],
                    rm[:uk, 2 * k + s: 2 * k + s + 1])
            xs.append(xk)
        if "dbg_xs0" in ap:
            nc.sync.dma_start(ap["dbg_xs0"][b], xs[0][:])
        return dict(s=stiles, xs=xs, g=[None] * 4)

    def em_g(b, pi, cur, copy_engs):
        """banded G matmuls for pair pi=(s,dd): two 1-bank psum tiles (per
        d-chunk of the speaker-masked x), copied into one sbuf tile
        [100, 600] (cols = ch*300 + t). fp8 for the DoubleRow second stage."""
        s, dd = PAIRS[pi]
        st = cur["s"][dd]
        g = gsb.tile([100, 2 * SEQ], F8, name=f"g{pi}", tag=f"g{pi}")
        for ch in range(2):
            pg = ps_g.tile([128, 512], F32, name="pg", tag="pg")
            for k, (u0, uk, c0, wkk) in enumerate(K):
                lhsT = cur["xs"][k][:uk, s * D + ch * 100: s * D + ch * 100 + 100]
                segs = _g_segments(k)
                last = (k == 2)
                for si, (j0, j1, stt) in enumerate(segs):
                    nc.tensor.matmul(
                        pg[:100, c0 + j0: c0 + j1],
                        lhsT, st[k][:uk, j0:j1],
                        start=(k == 0), stop=(last and si == len(segs) - 1))
            eng = copy_engs[ch]
            if eng is nc.scalar:
                nc.scalar.copy(g[:, ch * SEQ:(ch + 1) * SEQ], pg[:100, :SEQ])
            else:
                eng.tensor_copy(g[:, ch * SEQ:(ch + 1) * SEQ], pg[:100, :SEQ])
        if pi == 0 and "dbg_g0" in ap:
            nc.sync.dma_start(ap["dbg_g0"][b], g[:])
        return g

    DR = mybir.MatmulPerfMode.DoubleRow

    def em_ph1pair(b, pi, g, phA, phB, first, last):
        """accumulate pair pi's contribution into ph1 diff/base banks.
        fp8 DoubleRow: both 100-row d-chunks contract in one pass."""
        s, dd = PAIRS[pi]
        r4 = s * 2 + dd
        rhs = g.rearrange("p (c x) -> p c x", c=2)
        wd = c_w8d.rearrange("p (c x) -> p c x", c=2)[:, :, r4 * H:(r4 + 1) * H]
        wb = c_w8b.rearrange("p (c x) -> p c x", c=2)[:, :, r4 * H:(r4 + 1) * H]
        nc.tensor.matmul(phA[:H, :SEQ], wd, rhs, start=first, stop=last,
                         perf_mode=DR)
        nc.tensor.matmul(phB[:H, :SEQ], wb, rhs, start=False, stop=last,
                         perf_mode=DR)

    def em_proot(b, phB):
        for ch in range(2):
            nc.tensor.matmul(
                phB[:H, :SEQ], c_root[:, ch * H:(ch + 1) * H],
                xtc(ch)[:, b * SEQ:(b + 1) * SEQ],
                start=(ch == 0), stop=False)

    SELC = [(0, 128), (128, SEQ - 128)]

    def em_sel(b, phA, phB):
        """h1f = ph1_b + brc + tmb*ph1_d, split in two column chunks so the
        first qt matmul can start after the first chunk."""
        h1f = wk.tile([H, SEQ], BF, name="h1f", tag="h1f")
        t1 = wk.tile([H, SEQ], BF, name="t1", tag="t1")
        nc.vector.tensor_tensor(t1[:], c_tmb[:, b * SEQ:(b + 1) * SEQ],
                                phA[:H, :SEQ], op=OP.mult)
        nc.vector.scalar_tensor_tensor(
            h1f[:], phB[:H, :SEQ], c_brc[:], t1[:], op0=OP.add, op1=OP.add)
        if "dbg_h1f" in ap:
            nc.sync.dma_start(ap["dbg_h1f"][b], h1f[:])
        return h1f

    def em_tail_qt(t):
        b, h1f = t["b"], t["h1f"]
        pq = ps_b.tile([128, 512], F32, name="pq", tag="pb")
        for k, (u0, uk, c0, wkk) in enumerate(K):
            nc.tensor.matmul(pq[:uk, k * H:(k + 1) * H],
                             h1f[:, u0:u0 + uk], c_w2[:],
                             start=(k == 0), stop=(k == 2))
        qts = wk.tile([128, 3 * H], BF, name="qts", tag="qts")
        nc.vector.tensor_copy(qts[:], pq[:, :3 * H])
        if "dbg_qts" in ap:
            nc.sync.dma_start(ap["dbg_qts"][b], qts[:])
        t["qts"] = qts

    def em_tail_ph2(t):
        b, h1f, qts = t["b"], t["h1f"], t["qts"]
        p2 = ps_b.tile([128, 512], F32, name="p2", tag="pb")
        for o, w in SELC:
            nc.tensor.matmul(p2[:H, o:o + w], c_w1[:], h1f[:, o:o + w],
                             start=(o == 0), stop=False)
        for k, (u0, uk, c0, wkk) in enumerate(K):
            nc.tensor.matmul(p2[:H, c0:c0 + wkk], qts[:uk, k * H:(k + 1) * H],
                             c_dir["wnb"][:uk, k * WPAD:k * WPAD + wkk],
                             start=False, stop=(k == 2))
        h2 = wk.tile([H, SEQ], BF, name="h2", tag="h2")
        nc.scalar.activation(h2[:], p2[:H, :SEQ], AF.Identity, bias=c_bgc[:])
        if "dbg_h2" in ap:
            nc.sync.dma_start(ap["dbg_h2"][b], h2[:])
        t["h2"] = h2

    def em_tail_phid(t):
        b, h2 = t["b"], t["h2"]
        p3 = ps_b.tile([128, 512], F32, name="p3", tag="pb")
        for ch in range(2):
            nc.tensor.matmul(p3[:H, :SEQ], c_wlind[:, ch * H:(ch + 1) * H],
                             xtc(ch)[:, b * SEQ:(b + 1) * SEQ],
                             start=(ch == 0), stop=False)
        nc.tensor.matmul(p3[:H, :SEQ], c_wlinh[:], h2[:], start=False, stop=True)
        hid = wk.tile([H, SEQ], BF, name="hid", tag="hid")
        nc.scalar.activation(hid[:], p3[:H, :SEQ], AF.Relu, bias=c_blc[:])
        if "dbg_hid" in ap:
            nc.sync.dma_start(ap["dbg_hid"][b], hid[:])
        t["hid"] = hid

    def em_tail_plg(t):
        b, hid = t["b"], t["hid"]
        ci, bl = _chunk_of(b)
        pl = ps_b.tile([128, 512], F32, name="pl", tag="pb")
        for k, (u0, uk, c0, wkk) in enumerate(K):
            nc.tensor.matmul(pl[:uk, k * NCLS:(k + 1) * NCLS],
                             hid[:, u0:u0 + uk], c_wfc[:],
                             start=(k == 0), stop=False)
        nc.tensor.matmul(pl[:128, :3 * NCLS], c_ones[:1, :], c_bfc[:1, :],
                         start=False, stop=True)
        nc.scalar.copy(losb[ci][:, bl * 18:(bl + 1) * 18], pl[:, :3 * NCLS])
        if bl == CHUNKS[ci] - 1:
            em_lsm(ci)

    def em_lsm(ci):
        """batched log-softmax over one 8-dialogue chunk + output DMA."""
        NG = CHUNKS[ci] * 3
        l3 = losb[ci].rearrange("p (g c) -> p g c", c=NCLS)
        m = wk.tile([128, NG], BF, name="lm", tag="lm")
        nc.vector.reduce_max(m[:], l3, axis=AX.X)
        e = wk.tile([128, NG * NCLS], BF, name="le", tag="le")
        e3 = e.rearrange("p (g c) -> p g c", c=NCLS)
        for c in range(NCLS):
            nc.gpsimd.tensor_tensor(e3[:, :, c], l3[:, :, c], m[:], op=OP.subtract)
        ex = wk.tile([128, NG * NCLS], BF, name="lex", tag="lex")
        nc.scalar.activation(ex[:], e[:], AF.Exp)
        sm = wk.tile([128, NG], F32, name="lsum", tag="lsum")
        nc.vector.reduce_sum(sm[:], ex.rearrange("p (g c) -> p g c", c=NCLS),
                             axis=AX.X)
        lnz = wk.tile([128, NG], F32, name="lnz", tag="lnz")
        nc.scalar.activation(lnz[:], sm[:], AF.Ln)
        lsm = wk.tile([128, NG], F32, name="llsm", tag="llsm")
        nc.vector.tensor_tensor(lsm[:], m[:], lnz[:], op=OP.add)
        o3 = osb[ci].rearrange("p (g c) -> p g c", c=NCLS)
        for c in range(NCLS):
            nc.gpsimd.tensor_tensor(o3[:, :, c], l3[:, :, c], lsm[:],
                                    op=OP.subtract)
        o4 = osb[ci].rearrange("p (b k c) -> p b k c", b=CHUNKS[ci], k=3)
        for k, (u0, uk, c0, wkk) in enumerate(K):
            nc.sync.dma_start(
                ov[CHOFF[ci]:CHOFF[ci + 1], u0:u0 + uk, :].transpose([1, 0, 2]),
                o4[:uk, :, k, :])

    # ---------------- software-pipelined dialogue loop ----------------
    # Iteration b runs dialogue b's ph1/G mid-section, dialogue b+1's front,
    # and dialogue b-1's tail, interleaved so every vector-chain result has
    # PE work in front of it.

    SC, VE = nc.scalar, nc.vector

    GENG = [(SC, VE), (SC, VE), (SC, VE), (SC, SC)]
    cur = em_front(0)
    cur["g"][0] = em_g(0, 0, cur, GENG[0])
    cur["g"][1] = em_g(0, 1, cur, GENG[1])
    tail = None

    for b in range(BPC + 1):
        live = b < BPC
        if tail:
            em_tail_qt(tail)
        if live:
            phA = ps_b.tile([128, 512], F32, name="phA", tag="pb")
            phB = ps_b.tile([128, 512], F32, name="phB", tag="pb")
            em_proot(b, phB)
            em_ph1pair(b, 0, cur["g"][0], phA, phB, first=True, last=False)
        if tail:
            em_tail_ph2(tail)
        if live:
            cur["g"][2] = em_g(b, 2, cur, GENG[2])
            em_ph1pair(b, 1, cur["g"][1], phA, phB, first=False, last=False)
        if tail:
            em_tail_phid(tail)
        nxt = None
        if live:
            cur["g"][3] = em_g(b, 3, cur, GENG[3])
            if b + 1 < BPC:
                nxt = em_front(b + 1)
            em_ph1pair(b, 2, cur["g"][2], phA, phB, first=False, last=False)
            em_ph1pair(b, 3, cur["g"][3], phA, phB, first=False, last=True)
            h1f = em_sel(b, phA, phB)
            if nxt:
                nxt["g"][0] = em_g(b + 1, 0, nxt, GENG[0])
                nxt["g"][1] = em_g(b + 1, 1, nxt, GENG[1])
        if tail:
            em_tail_plg(tail)
        tail = dict(b=b, h1f=h1f) if live else None
        cur = nxt


def _host_prep(inputs):
    feats = np.asarray(inputs["features"], dtype=np.float32)    # (300,256,200)
    spk = np.asarray(inputs["speakers"])                        # (300,256)
    W_att = np.asarray(inputs["W_att"], dtype=np.float32)
    basis = np.asarray(inputs["basis"], dtype=np.float32)
    comp = np.asarray(inputs["comp"], dtype=np.float32)
    root = np.asarray(inputs["root"], dtype=np.float32)
    bias_r = np.asarray(inputs["bias_r"], dtype=np.float32)
    W1 = np.asarray(inputs["W1"], dtype=np.float32)
    W2 = np.asarray(inputs["W2"], dtype=np.float32)
    b_gc = np.asarray(inputs["b_gc"], dtype=np.float32)
    W_lin = np.asarray(inputs["W_lin"], dtype=np.float32)
    b_lin = np.asarray(inputs["b_lin"], dtype=np.float32)
    W_fc = np.asarray(inputs["W_fc"], dtype=np.float32)
    b_fc = np.asarray(inputs["b_fc"], dtype=np.float32)

    i = np.arange(SEQ)[:, None]
    j = np.arange(SEQ)[None, :]
    win = (j >= i - WP) & (j <= i + WF)
    dir0 = (win & (i < j))
    dir1 = (win & (i >= j))

    def banded(m):
        outm = np.zeros((3, 128, WPAD), np.float32)
        for k, (u0, uk, c0, wkk) in enumerate(K):
            outm[k, :uk, :wkk] = m[u0:u0 + uk, c0:c0 + wkk]
        return outm.astype(BF_NP)

    w8 = np.einsum("rb,bdh->rdh", comp, basis).astype(np.float32)  # (8,200,H)
    w8r = w8.reshape(2, 2, 2, 2, 100, H)      # (s, tau, dd, ch, 100, H)
    w8_d = (w8r[:, 0] - w8r[:, 1])            # (s, dd, ch, 100, H)
    w8_b = w8r[:, 1]
    # -> (ch, 100, (s*2+dd)*H + h); the h1 path carries a x{H1S} scale so the
    # fp8e4 weights sit in normal range; W1/W2 absorb the inverse.
    w8d = np.ascontiguousarray(
        w8_d.transpose(2, 3, 0, 1, 4).reshape(2, 100, 4 * H) * H1S).astype(F8_NP)
    w8b = np.ascontiguousarray(
        w8_b.transpose(2, 3, 0, 1, 4).reshape(2, 100, 4 * H) * H1S).astype(F8_NP)

    shared = {
        "wattc": np.ascontiguousarray(W_att.reshape(2, 100, SEQ)).astype(BF_NP),
        "w8d": w8d, "w8b": w8b,
        "rootc": np.ascontiguousarray(
            (root * H1S).reshape(2, 100, H).transpose(1, 0, 2).reshape(100, 2 * H)
        ).astype(BF_NP),
        "wlindc": np.ascontiguousarray(
            W_lin[:D].reshape(2, 100, H).transpose(1, 0, 2).reshape(100, 2 * H)
        ).astype(BF_NP),
        "w1c": (W1 / H1S).astype(BF_NP), "w2c": (W2 / H1S).astype(BF_NP),
        "wlinhc": np.ascontiguousarray(W_lin[D:]).astype(BF_NP),
        "wfcc": W_fc.astype(BF_NP),
        "onesc": np.ones((1, H), BF_NP),
        "bfcrow": np.tile(b_fc, 3)[None, :].astype(BF_NP),
        "d0b": banded(dir0.astype(np.float32)),
        "d1b": banded(dir1.astype(np.float32)),
        "wnb": banded(win.astype(np.float32)),
        "brc": (bias_r * H1S).reshape(H, 1), "bgc": b_gc.reshape(H, 1),
        "blc": b_lin.reshape(H, 1),
    }

    in_maps = []
    for c in range(NCORES):
        bs = slice(c * BPC, (c + 1) * BPC)
        fb = feats[:, bs, :]                                    # (300,32,200)
        sp = spk[:, bs]                                         # (300,32)
        xt2 = np.ascontiguousarray(
            fb.transpose(2, 1, 0).reshape(2, 100, BPC * SEQ)).astype(BF_NP)
        m = {"xt2": xt2}
        for k, (u0, uk, c0, wkk) in enumerate(K):
            xn = np.zeros((128 if k < 2 else 44, BPC * D), np.float32)
            xn[:uk] = fb[u0:u0 + uk].reshape(uk, BPC * D)
            m[f"xn{k}"] = xn.astype(BF_NP)
        mskp = np.zeros((128, BPC * 6), np.float32)
        for k, (u0, uk, c0, wkk) in enumerate(K):
            spl = sp[u0:u0 + uk]                                # (uk, 32)
            for s in range(2):
                mskp[:uk, np.arange(BPC) * 6 + 2 * k + s] = (spl == s)
        m["mskp"] = mskp
        m["tmb"] = (sp.T == 0).astype(BF_NP)                    # (32, 300)
        m.update(shared)
        in_maps.append(m)
    return in_maps


def get_program():
    if "nc" not in _CACHE:
        _CACHE["nc"] = _build_program()
    return _CACHE["nc"]


def kernel(**inputs):
    nc = get_program()
    in_maps = _host_prep(inputs)
    res = bass_utils.run_bass_kernel_spmd(nc, in_maps, core_ids=list(range(NCORES)))
    if os.environ.get("BASS_DEBUG_TAPS"):
        kernel.debug = res.results
    return np.concatenate([res.results[c]["out"] for c in range(NCORES)], axis=0)
